# revision 2
# baseline (speedup 1.0000x reference)
"""Trainium2 Bass kernel for nn_ContrastivePredictionLoss.

Reference computation (B=64, feat = 4*256*256 = 262144):
    errors[b] = mean |pred_mean[b] - targets[b]|        (per-sample, heavy)
    unc[b]    = mean pred_std[b]                        (per-sample, heavy)
    loss      = sum_{i<j} relu(where(e_i>e_j, u_j-u_i, u_i-u_j) + 1) / npairs

Strategy (8 NeuronCores, data-parallel on batch):
  - Each core streams its 8 samples (3 x 8 MiB) through SBUF in
    [128, 4096] tiles (2 samples per tile: partitions 0-63 -> sample 2t,
    64-127 -> sample 2t+1) and computes per-partition partial sums on DVE.
  - A tiny PE matmul with a block-indicator matrix (scaled by 1/feat,
    exact: feat = 2^18) turns partials into per-sample means.
  - AllGather (64 B/core) replicates all errors/uncertainties; each core
    computes the pairwise hinge loss on the [64,64] matrix.

Pairwise identity used on device: the pair matrix
    D[i,j] = where(e_i>e_j, u_j-u_i, u_i-u_j) + m
           = m - sign(e_j-e_i)*(u_j-u_i)
is symmetric (for non-tied errors), and D[i,i] = m, so
    sum_{i<j} relu(D) = (sum_{all i,j} relu(D) - B*m) / 2.
"""

import numpy as np
from contextlib import ExitStack

import concourse.bass as bass
import concourse.bacc as bacc
import concourse.mybir as mybir
import concourse.tile as tile
from concourse.bass_utils import run_bass_kernel_spmd

N_CORES = 8
B = 64
B_LOC = B // N_CORES          # 8 samples per core
FEAT = 4 * 256 * 256          # 262144 = 2^18
MARGIN = 1.0
NUM_PAIRS = B * (B - 1) // 2  # 2016

F32 = mybir.dt.float32


def build_nc(feat: int = FEAT):
    """Build + compile the 8-core Bass program.

    feat must be divisible by 64; tiles are [128, feat//64] (2 samples per
    tile), N_TILES = B_LOC//2 = 4 tiles per tensor per core.
    """
    assert feat % 64 == 0
    tile_f = feat // 64        # free dim of one [128, tile_f] tile
    n_tiles = B_LOC // 2       # 4
    inv_feat = 1.0 / feat

    nc = bacc.Bacc(
        "TRN2",
        target_bir_lowering=False,
        debug=False,
        num_devices=N_CORES,
    )

    pm = nc.dram_tensor("pred_mean", [n_tiles, 128, tile_f], F32, kind="ExternalInput")
    tg = nc.dram_tensor("targets", [n_tiles, 128, tile_f], F32, kind="ExternalInput")
    st = nc.dram_tensor("pred_std", [n_tiles, 128, tile_f], F32, kind="ExternalInput")
    out = nc.dram_tensor("out", [1], F32, kind="ExternalOutput")

    with tile.TileContext(nc) as tc, ExitStack() as ctx:
        io = ctx.enter_context(tc.tile_pool(name="io", bufs=2))
        work = ctx.enter_context(tc.tile_pool(name="work", bufs=2))
        small = ctx.enter_context(tc.tile_pool(name="small", bufs=1))
        psum = ctx.enter_context(
            tc.tile_pool(name="psum", bufs=1, space=bass.MemorySpace.PSUM)
        )
        dram = ctx.enter_context(
            tc.tile_pool(name="dram", bufs=1, space=bass.MemorySpace.DRAM)
        )

        # ---- per-core reductions: acc[:, t] = err partials, acc[:, 4+t] = unc
        acc = small.tile([128, 2 * n_tiles], F32)
        for t in range(n_tiles):
            a = io.tile([128, tile_f], F32, tag="a")
            b_ = io.tile([128, tile_f], F32, tag="b")
            s_ = io.tile([128, tile_f], F32, tag="s")
            nc.sync.dma_start(out=a[:], in_=pm[t])
            nc.sync.dma_start(out=b_[:], in_=tg[t])
            nc.sync.dma_start(out=s_[:], in_=st[t])
            d = work.tile([128, tile_f], F32, tag="d")
            nc.vector.tensor_sub(d[:], a[:], b_[:])
            nc.vector.tensor_reduce(
                acc[:, t : t + 1],
                d[:],
                axis=mybir.AxisListType.X,
                op=mybir.AluOpType.add,
                apply_absolute_value=True,
            )
            nc.vector.tensor_reduce(
                acc[:, n_tiles + t : n_tiles + t + 1],
                s_[:],
                axis=mybir.AxisListType.X,
                op=mybir.AluOpType.add,
            )

        # ---- per-sample means via block-indicator matmul (scaled by 1/feat)
        # ind[p, c] = inv_feat if (p < 64) == (c == 0) else 0
        ind = small.tile([128, 2], F32)
        nc.vector.memset(ind[:], 0.0)
        nc.vector.memset(ind[0:64, 0:1], inv_feat)
        nc.vector.memset(ind[64:128, 1:2], inv_feat)
        means_ps = psum.tile([2, 2 * n_tiles], F32)
        # means_ps[c, t] = err mean of local sample 2t+c, [c, 4+t] = unc mean
        nc.tensor.matmul(means_ps[:], ind[:], acc[:], start=True, stop=True)

        # ---- pack means into DRAM as [row(err/unc), t, c]; local sample 2t+c
        means_sb = small.tile([2, 2 * n_tiles], F32)
        nc.vector.tensor_copy(means_sb[:], means_ps[:])
        ag_in = dram.tile([2, n_tiles, 2], F32)
        nc.sync.dma_start(
            out=ag_in[:].rearrange("row t c -> c row t"), in_=means_sb[:]
        )

        # ---- allgather: out[r, row, t, c]; global sample b = r*8 + 2t + c
        ag_out = dram.tile([N_CORES, 2, n_tiles, 2], F32)
        nc.gpsimd.collective_compute(
            "AllGather",
            mybir.AluOpType.bypass,
            replica_groups=[list(range(N_CORES))],
            ins=[ag_in[:]],
            outs=[ag_out[:]],
        )

        # ---- replicated pairwise hinge loss on [64, 64]
        err_prt = small.tile([B, 1], F32)   # err_b on partition b
        unc_prt = small.tile([B, 1], F32)
        rowv = small.tile([1, 2 * B], F32)  # [err(64), unc(64)] on one partition
        nc.sync.dma_start(out=err_prt[:], in_=ag_out[:, 0])
        nc.sync.dma_start(out=unc_prt[:], in_=ag_out[:, 1])
        nc.sync.dma_start(out=rowv[0:1, 0:B], in_=ag_out[:, 0])
        nc.sync.dma_start(out=rowv[0:1, B : 2 * B], in_=ag_out[:, 1])

        ones_row = small.tile([1, B], F32)
        nc.vector.memset(ones_row[:], 1.0)
        bcast = psum.tile([B, 2 * B], F32)
        # bcast[p, q] = rowv[q] for every partition p
        nc.tensor.matmul(bcast[:], ones_row[:], rowv[:], start=True, stop=True)

        de = small.tile([B, B], F32)
        du = small.tile([B, B], F32)
        nc.vector.tensor_scalar_sub(de[:], bcast[:, 0:B], err_prt[:])       # e_q - e_p
        nc.vector.tensor_scalar_sub(du[:], bcast[:, B : 2 * B], unc_prt[:])  # u_q - u_p
        sgn = small.tile([B, B], F32)
        nc.scalar.sign(sgn[:], de[:])
        prod = small.tile([B, B], F32)
        nc.vector.tensor_mul(prod[:], sgn[:], du[:])
        hinge = small.tile([B, B], F32)
        rows = small.tile([B, 1], F32)
        # hinge = relu(m - prod), rows = per-partition sum
        nc.scalar.activation(
            hinge[:],
            prod[:],
            mybir.ActivationFunctionType.Relu,
            bias=MARGIN,
            scale=-1.0,
            accum_out=rows[:],
        )
        ones_col = small.tile([B, 1], F32)
        nc.vector.memset(ones_col[:], 1.0)
        total_ps = psum.tile([1, 1], F32)
        nc.tensor.matmul(total_ps[:], ones_col[:], rows[:], start=True, stop=True)
        loss_sb = small.tile([1, 1], F32)
        scale = 1.0 / (2 * NUM_PAIRS)
        nc.scalar.activation(
            loss_sb[:],
            total_ps[:],
            mybir.ActivationFunctionType.Copy,
            bias=-B * MARGIN * scale,
            scale=scale,
        )
        nc.sync.dma_start(out=out[:], in_=loss_sb[:])

    nc.compile()
    return nc


def shard_inputs(pred_mean, pred_std, targets, feat: int = FEAT):
    tile_f = feat // 64
    n_tiles = B_LOC // 2
    in_maps = []
    for r in range(N_CORES):
        sl = slice(r * B_LOC, (r + 1) * B_LOC)
        in_maps.append(
            {
                "pred_mean": np.ascontiguousarray(pred_mean[sl], dtype=np.float32).reshape(
                    n_tiles, 128, tile_f
                ),
                "targets": np.ascontiguousarray(targets[sl], dtype=np.float32).reshape(
                    n_tiles, 128, tile_f
                ),
                "pred_std": np.ascontiguousarray(pred_std[sl], dtype=np.float32).reshape(
                    n_tiles, 128, tile_f
                ),
            }
        )
    return in_maps


_NC_CACHE = {}


def _get_nc():
    if "nc" not in _NC_CACHE:
        _NC_CACHE["nc"] = build_nc()
    return _NC_CACHE["nc"]


def kernel(pred_mean, pred_std, targets):
    nc = _get_nc()
    in_maps = shard_inputs(pred_mean, pred_std, targets)
    res = run_bass_kernel_spmd(nc, in_maps, core_ids=list(range(N_CORES)))
    loss = res.results[0]["out"][0]
    return np.asarray(loss, dtype=np.float32).reshape(())
